# revision 1
# baseline (speedup 1.0000x reference)
"""Trainium2 Bass kernel for AdaptiveRankFusionLayer (CP low-rank fusion).

    out = ((x1 @ f1) * (x2 @ f2) * (x3 @ f3)) @ f_out.T

Data-parallel batch sharding across 8 NeuronCores (65536 -> 8192
rows/core), no collectives. bf16 compute (inputs cast f32->bf16 during
the SWDGE load DMA), fp32 PSUM accumulation, fp32 output.

Per 512-row supertile:
  1. gpsimd cast-DMA x1/x2/x3 tiles -> SBUF bf16, natural layout
     [128 batch, feat].
  2. Transpose 128x128 blocks to get xT [feat, batch]:
     PE is_transpose (identity-stationary) -> PSUM -> copy to SBUF,
     or DMA xbar transpose SBUF->SBUF for a fraction (XBAR_FRAC).
  3. Skinny matmuls accumulate y_iT [10, 512] = f_i.T @ x_i.T in PSUM.
  4. Hadamard product -> y [10, 512] bf16 in SBUF.
  5. Final matmul out[128b, 512] = y_chunk.T @ f_outT per batch block.
  6. Copy PSUM->SBUF f32, DMA out.
"""

import sys
import types

import numpy as np

import concourse.bass as bass
import concourse.mybir as mybir
import concourse.tile as tile
from concourse import bacc
from concourse.bass_utils import run_bass_kernel_spmd
from concourse.masks import make_identity


def _install_profile_shim():
    """Make trace=True / BASS_TRACE=1 work in this container: provide the
    antenv.axon_hooks module the axon NTFF-profile path imports, and make
    artifact upload a no-op (no object store here). Safe no-op if the real
    module exists."""
    try:
        if "antenv.axon_hooks" not in sys.modules:
            try:
                import antenv.axon_hooks  # noqa: F401
            except ImportError:
                mod = types.ModuleType("antenv.axon_hooks")
                mod._hook = None
                mod.set_axon_ntff_profile_hook = (
                    lambda h: setattr(mod, "_hook", h))
                mod.get_axon_ntff_profile_hook = lambda: mod._hook
                sys.modules["antenv.axon_hooks"] = mod
                import antenv
                antenv.axon_hooks = mod
                try:
                    from trn_agent_boot.trn_boot import (
                        _ntff_profile_via_ctypes)
                    mod.set_axon_ntff_profile_hook(
                        _ntff_profile_via_ctypes("/opt/axon/libaxon_pjrt.so"))
                except Exception:
                    pass
        import concourse.bass_utils as _bu
        _orig_upload = _bu.upload_artifacts

        def _safe_upload(tmpdir):
            try:
                return _orig_upload(tmpdir)
            except Exception:
                return f"file://{tmpdir}"

        _bu.upload_artifacts = _safe_upload
    except Exception:
        pass


_install_profile_shim()

N_CORES = 8
B = 65536
B_LOCAL = B // N_CORES
SIZES = (1024, 512, 768)
OUT = 512
RANK = 10
SUPER = 512  # batch rows per supertile
F32 = mybir.dt.float32
BF16 = mybir.dt.bfloat16

# fraction of 128x128 transpose blocks routed to DMA xbar instead of PE
XBAR_EVERY = 0  # 0 = none; n>0 = every n-th k-tile goes to xbar


def build(b_local=B_LOCAL, xbar_every=XBAR_EVERY):
    nsup = b_local // SUPER
    kts = [f // 128 for f in SIZES]  # k-tiles per input: 8, 4, 6

    nc = bacc.Bacc("TRN2", target_bir_lowering=False, debug=False,
                   num_devices=N_CORES)
    x_dram = [
        nc.dram_tensor(f"x{i+1}", (b_local, SIZES[i]), F32,
                       kind="ExternalInput").ap()
        for i in range(3)
    ]
    f_dram = [
        nc.dram_tensor(f"f{i+1}", (SIZES[i], RANK), F32,
                       kind="ExternalInput").ap()
        for i in range(3)
    ]
    fo_dram = nc.dram_tensor("f_out", (OUT, RANK), F32,
                             kind="ExternalInput").ap()
    out_dram = nc.dram_tensor("out", (b_local, OUT), F32,
                              kind="ExternalOutput").ap()

    with tile.TileContext(nc) as tc:
        with (
            tc.tile_pool(name="const", bufs=1) as constp,
            tc.tile_pool(name="xin", bufs=3) as xinp,
            tc.tile_pool(name="xt", bufs=6) as xtp,
            tc.tile_pool(name="ysb", bufs=2) as yp,
            tc.tile_pool(name="osb", bufs=2) as osp,
            tc.tile_pool(name="pst", bufs=2, space="PSUM") as pst,
            tc.tile_pool(name="psy", bufs=1, space="PSUM") as psy,
            tc.tile_pool(name="pso", bufs=2, space="PSUM") as pso,
        ):
            # identity for PE transposes
            ident = constp.tile([128, 128], BF16)
            make_identity(nc, ident[:])

            # factor matrices, natural layout [128 feat, kt, rank], bf16
            f_sb = []
            for i in range(3):
                t = constp.tile([128, kts[i], RANK], BF16, tag=f"f{i}",
                                name=f"f_sb{i}")
                nc.gpsimd.dma_start(
                    t[:], f_dram[i].rearrange("(kt p) r -> p kt r", p=128))
                f_sb.append(t)

            # f_outT [10, 512] bf16 via 4 PE transposes
            fo_sb = constp.tile([128, 4, RANK], BF16, tag="fo")
            nc.gpsimd.dma_start(
                fo_sb[:], fo_dram.rearrange("(blk p) r -> p blk r", p=128))
            foT = constp.tile([RANK, 4, 128], BF16, tag="foT")
            for blk in range(4):
                pt = pso.tile([RANK, 128], BF16, tag="ops",
                              name=f"fotps{blk}")
                nc.tensor.transpose(pt[:], fo_sb[:, blk, :], ident[:])
                nc.scalar.copy(foT[:, blk, :], pt[:])

            toggle = 0
            for s in range(nsup):
                # load supertile: [128 part, 4 blk, feat], cast f32->bf16
                x_t = []
                for i in range(3):
                    t = xinp.tile([128, 4, SIZES[i]], BF16, tag=f"x{i}",
                                  name=f"x_t{i}_{s}")
                    src = x_dram[i].rearrange(
                        "(s blk p) f -> s p blk f", blk=4, p=128)[s]
                    nc.gpsimd.dma_start(t[:], src)
                    x_t.append(t)

                # transposes + k-matmuls
                y_ps = [psy.tile([RANK, SUPER], F32, tag=f"y{i}",
                                 name=f"y_ps{i}_{s}")
                        for i in range(3)]
                ktglobal = 0
                for i in range(3):
                    for kt in range(kts[i]):
                        xT_sb = xtp.tile([128, SUPER], BF16, tag="xtsb")
                        use_xbar = (xbar_every and
                                    ktglobal % xbar_every == xbar_every - 1)
                        if use_xbar:
                            for blk in range(4):
                                eng = nc.sync if blk % 2 == 0 else nc.scalar
                                eng.dma_start_transpose(
                                    xT_sb[:, blk * 128:(blk + 1) * 128],
                                    x_t[i][:, blk, kt * 128:(kt + 1) * 128])
                        else:
                            # transpose as a REGULAR matmul against a
                            # streamed identity: x_blk.T @ I. Unlike
                            # is_transpose, these pipeline back-to-back,
                            # use FWL for the bf16 weight load, and warm
                            # the HAM clock gate.
                            xT_ps = pst.tile([128, SUPER], F32, tag="xtps")
                            for blk in range(4):
                                nc.tensor.matmul(
                                    xT_ps[:, blk * 128:(blk + 1) * 128],
                                    x_t[i][:, blk, kt * 128:(kt + 1) * 128],
                                    ident[:],
                                    start=True, stop=True)
                            if toggle % 2 == 0:
                                nc.vector.tensor_copy(xT_sb[:], xT_ps[:])
                            else:
                                nc.scalar.copy(xT_sb[:], xT_ps[:])
                            toggle += 1
                        ktglobal += 1
                        nc.tensor.matmul(
                            y_ps[i][:], f_sb[i][:, kt, :], xT_sb[:],
                            start=(kt == 0), stop=(kt == kts[i] - 1))

                # hadamard (only one PSUM operand per tensor_tensor)
                y2_sb = yp.tile([RANK, SUPER], BF16, tag="y2sb")
                nc.scalar.copy(y2_sb[:], y_ps[1][:])
                y_sb = yp.tile([RANK, SUPER], BF16, tag="ysb")
                nc.vector.tensor_mul(y_sb[:], y_ps[0][:], y2_sb[:])
                nc.vector.tensor_mul(y_sb[:], y_sb[:], y_ps[2][:])

                # final matmuls
                o_sb = osp.tile([128, 4, OUT], F32, tag="osb")
                for blk in range(4):
                    o_ps = pso.tile([128, OUT], F32, tag="ops",
                                    name=f"o_ps_{s}_{blk}")
                    nc.tensor.matmul(
                        o_ps[:], y_sb[:, blk * 128:(blk + 1) * 128], foT[:],
                        start=True, stop=True)
                    if blk % 2 == 0:
                        nc.scalar.copy(o_sb[:, blk, :], o_ps[:])
                    else:
                        nc.vector.tensor_copy(o_sb[:, blk, :], o_ps[:])
                dst = out_dram.rearrange(
                    "(s blk p) o -> s p blk o", blk=4, p=128)[s]
                nc.sync.dma_start(dst, o_sb[:])

    nc.compile()
    return nc


_NC_CACHE = {}


def _get_nc(b_local=B_LOCAL):
    if b_local not in _NC_CACHE:
        _NC_CACHE[b_local] = build(b_local)
    return _NC_CACHE[b_local]


def kernel(x1, x2, x3, f1, f2, f3, f_out):
    nc = _get_nc()
    in_maps = []
    for c in range(N_CORES):
        sl = slice(c * B_LOCAL, (c + 1) * B_LOCAL)
        in_maps.append({
            "x1": np.ascontiguousarray(x1[sl]),
            "x2": np.ascontiguousarray(x2[sl]),
            "x3": np.ascontiguousarray(x3[sl]),
            "f1": f1, "f2": f2, "f3": f3, "f_out": f_out,
        })
    res = run_bass_kernel_spmd(nc, in_maps, core_ids=list(range(N_CORES)))
    return np.concatenate([res.results[c]["out"] for c in range(N_CORES)],
                          axis=0)


# revision 7
# speedup vs baseline: 1.0263x; 1.0263x over previous
"""Trainium2 Bass kernel for AdaptiveRankFusionLayer (CP low-rank fusion).

    out = ((x1 @ f1) * (x2 @ f2) * (x3 @ f3)) @ f_out.T

Data-parallel batch sharding across 8 NeuronCores (65536 -> 8192
rows/core), no collectives. bf16 compute (inputs cast f32->bf16 during
the SWDGE load DMA), fp32 PSUM accumulation, fp32 output.

Per 512-row supertile:
  1. gpsimd cast-DMA x1/x2/x3 tiles -> SBUF bf16, natural layout
     [128 batch, feat].
  2. Transpose 128x128 blocks to get xT [feat, batch]:
     PE is_transpose (identity-stationary) -> PSUM -> copy to SBUF,
     or DMA xbar transpose SBUF->SBUF for a fraction (XBAR_FRAC).
  3. Skinny matmuls accumulate y_iT [10, 512] = f_i.T @ x_i.T in PSUM.
  4. Hadamard product -> y [10, 512] bf16 in SBUF.
  5. Final matmul out[128b, 512] = y_chunk.T @ f_outT per batch block.
  6. Copy PSUM->SBUF f32, DMA out.
"""

import sys
import types

import numpy as np

import concourse.bass as bass
import concourse.mybir as mybir
import concourse.tile as tile
from concourse import bacc
from concourse.bass_utils import run_bass_kernel_spmd
from concourse.masks import make_identity


def _install_profile_shim():
    """Make trace=True / BASS_TRACE=1 work in this container: provide the
    antenv.axon_hooks module the axon NTFF-profile path imports, and make
    artifact upload a no-op (no object store here). Safe no-op if the real
    module exists."""
    try:
        if "antenv.axon_hooks" not in sys.modules:
            try:
                import antenv.axon_hooks  # noqa: F401
            except ImportError:
                mod = types.ModuleType("antenv.axon_hooks")
                mod._hook = None
                mod.set_axon_ntff_profile_hook = (
                    lambda h: setattr(mod, "_hook", h))
                mod.get_axon_ntff_profile_hook = lambda: mod._hook
                sys.modules["antenv.axon_hooks"] = mod
                import antenv
                antenv.axon_hooks = mod
                try:
                    from trn_agent_boot.trn_boot import (
                        _ntff_profile_via_ctypes)
                    mod.set_axon_ntff_profile_hook(
                        _ntff_profile_via_ctypes("/opt/axon/libaxon_pjrt.so"))
                except Exception:
                    pass
        import concourse.bass_utils as _bu
        _orig_upload = _bu.upload_artifacts

        def _safe_upload(tmpdir):
            try:
                return _orig_upload(tmpdir)
            except Exception:
                return f"file://{tmpdir}"

        _bu.upload_artifacts = _safe_upload
    except Exception:
        pass


_install_profile_shim()

N_CORES = 8
B = 65536
B_LOCAL = B // N_CORES
SIZES = (1024, 512, 768)
OUT = 512
RANK = 10
SUPER = 512  # batch rows per supertile
F32 = mybir.dt.float32
BF16 = mybir.dt.bfloat16

# fraction of 128x128 transpose blocks routed to DMA xbar instead of PE
XBAR_EVERY = 0  # 0 = none; n>0 = every n-th k-tile goes to xbar


def build(b_local=B_LOCAL, xbar_every=XBAR_EVERY):
    nsup = b_local // SUPER
    kts = [f // 128 for f in SIZES]  # k-tiles per input: 8, 4, 6

    nc = bacc.Bacc("TRN2", target_bir_lowering=False, debug=False,
                   num_devices=N_CORES)
    x_dram = [
        nc.dram_tensor(f"x{i+1}", (b_local, SIZES[i]), F32,
                       kind="ExternalInput").ap()
        for i in range(3)
    ]
    f_dram = [
        nc.dram_tensor(f"f{i+1}", (SIZES[i], RANK), F32,
                       kind="ExternalInput").ap()
        for i in range(3)
    ]
    fo_dram = nc.dram_tensor("f_out", (OUT, RANK), F32,
                             kind="ExternalInput").ap()
    out_dram = nc.dram_tensor("out", (b_local, OUT), F32,
                              kind="ExternalOutput").ap()

    with tile.TileContext(nc) as tc:
        with (
            tc.tile_pool(name="const", bufs=1) as constp,
            tc.tile_pool(name="xin", bufs=3) as xinp,
            tc.tile_pool(name="xt", bufs=6) as xtp,
            tc.tile_pool(name="ysb", bufs=2) as yp,
            tc.tile_pool(name="osb", bufs=2) as osp,
            tc.tile_pool(name="pst", bufs=2, space="PSUM") as pst,
            tc.tile_pool(name="psy", bufs=1, space="PSUM") as psy,
            tc.tile_pool(name="pso", bufs=2, space="PSUM") as pso,
        ):
            # identity for PE transposes
            ident = constp.tile([128, 128], BF16)
            make_identity(nc, ident[:])

            # factor matrices, natural layout [128 feat, kt, rank], bf16
            f_sb = []
            for i in range(3):
                t = constp.tile([128, kts[i], RANK], BF16, tag=f"f{i}",
                                name=f"f_sb{i}")
                nc.gpsimd.dma_start(
                    t[:], f_dram[i].rearrange("(kt p) r -> p kt r", p=128))
                f_sb.append(t)

            # f_outT [10, 512] bf16 via 4 PE transposes
            fo_sb = constp.tile([128, 4, RANK], BF16, tag="fo")
            nc.gpsimd.dma_start(
                fo_sb[:], fo_dram.rearrange("(blk p) r -> p blk r", p=128))
            foT = constp.tile([RANK, 4, 128], BF16, tag="foT")
            for blk in range(4):
                pt = pso.tile([RANK, 128], BF16, tag="ops",
                              name=f"fotps{blk}")
                nc.tensor.transpose(pt[:], fo_sb[:, blk, :], ident[:])
                nc.scalar.copy(foT[:, blk, :], pt[:])

            toggle = 0
            for s in range(nsup):
                # load supertile: [128 part, 4 blk, feat], cast f32->bf16
                x_t = []
                for i in range(3):
                    t = xinp.tile([128, 4, SIZES[i]], BF16, tag=f"x{i}",
                                  name=f"x_t{i}_{s}")
                    src = x_dram[i].rearrange(
                        "(s blk p) f -> s p blk f", blk=4, p=128)[s]
                    nc.gpsimd.dma_start(t[:], src)
                    x_t.append(t)

                # transposes + k-matmuls
                y_ps = [psy.tile([RANK, SUPER], F32, tag=f"y{i}",
                                 name=f"y_ps{i}_{s}")
                        for i in range(3)]
                ktglobal = 0
                for i in range(3):
                    for kt in range(kts[i]):
                        xT_sb = xtp.tile([128, SUPER], BF16, tag="xtsb")
                        use_xbar = (xbar_every and
                                    ktglobal % xbar_every == xbar_every - 1)
                        if use_xbar:
                            for blk in range(4):
                                eng = nc.sync if blk % 2 == 0 else nc.scalar
                                eng.dma_start_transpose(
                                    xT_sb[:, blk * 128:(blk + 1) * 128],
                                    x_t[i][:, blk, kt * 128:(kt + 1) * 128])
                        else:
                            # transpose as a REGULAR matmul against a
                            # streamed identity: x_blk.T @ I. Unlike
                            # is_transpose, these pipeline back-to-back,
                            # use FWL for the bf16 weight load, and warm
                            # the HAM clock gate.
                            xT_ps = pst.tile([128, SUPER], F32, tag="xtps")
                            for blk in range(4):
                                nc.tensor.matmul(
                                    xT_ps[:, blk * 128:(blk + 1) * 128],
                                    x_t[i][:, blk, kt * 128:(kt + 1) * 128],
                                    ident[:],
                                    start=True, stop=True)
                            if toggle % 2 == 0:
                                nc.vector.tensor_copy(xT_sb[:], xT_ps[:])
                            else:
                                nc.scalar.copy(xT_sb[:], xT_ps[:])
                            toggle += 1
                        ktglobal += 1
                        nc.tensor.matmul(
                            y_ps[i][:], f_sb[i][:, kt, :], xT_sb[:],
                            start=(kt == 0), stop=(kt == kts[i] - 1))

                # hadamard (only one PSUM operand per tensor_tensor)
                y2_sb = yp.tile([RANK, SUPER], BF16, tag="y2sb")
                nc.scalar.copy(y2_sb[:], y_ps[1][:])
                y_sb = yp.tile([RANK, SUPER], BF16, tag="ysb")
                nc.vector.tensor_mul(y_sb[:], y_ps[0][:], y2_sb[:])
                nc.vector.tensor_mul(y_sb[:], y_sb[:], y_ps[2][:])

                # final matmuls
                o_sb = osp.tile([128, 4, OUT], F32, tag="osb")
                for blk in range(4):
                    o_ps = pso.tile([128, OUT], F32, tag="ops",
                                    name=f"o_ps_{s}_{blk}")
                    nc.tensor.matmul(
                        o_ps[:], y_sb[:, blk * 128:(blk + 1) * 128], foT[:],
                        start=True, stop=True)
                    if blk % 2 == 0:
                        nc.scalar.copy(o_sb[:, blk, :], o_ps[:])
                    else:
                        nc.vector.tensor_copy(o_sb[:, blk, :], o_ps[:])
                dst = out_dram.rearrange(
                    "(s blk p) o -> s p blk o", blk=4, p=128)[s]
                nc.sync.dma_start(dst, o_sb[:])

    nc.compile()
    return nc


_NC_CACHE = {}


def _get_nc(b_local=B_LOCAL):
    if b_local not in _NC_CACHE:
        _NC_CACHE[b_local] = build(b_local)
    return _NC_CACHE[b_local]


def kernel(x1, x2, x3, f1, f2, f3, f_out):
    nc = _get_nc()
    in_maps = []
    for c in range(N_CORES):
        sl = slice(c * B_LOCAL, (c + 1) * B_LOCAL)
        in_maps.append({
            "x1": np.ascontiguousarray(x1[sl]),
            "x2": np.ascontiguousarray(x2[sl]),
            "x3": np.ascontiguousarray(x3[sl]),
            "f1": f1, "f2": f2, "f3": f3, "f_out": f_out,
        })
    res = run_bass_kernel_spmd(nc, in_maps, core_ids=list(range(N_CORES)))
    return np.concatenate([res.results[c]["out"] for c in range(N_CORES)],
                          axis=0)
